# revision 51
# baseline (speedup 1.0000x reference)
"""Trainium2 8-core kernel for an attention block (per-head full-width QKV).

Reference computation (B=2, S=2048, H=12, D=768):
    Q/K/V = einsum('bsd,hde->bhse', x, W_{q,k,v})      # per-head D->D projections
    attn  = causal softmax(Q K^T / sqrt(D)) @ V
    out   = concat_heads(attn) @ W_o.T                 # [B,S,D]
    out   = out + gelu(LN(out) @ ff_w1.T) @ ff_w2.T

Sharding over 8 cores: 2 batch groups x 4 ranks. Core c = 4*b + r handles
batch b and heads [3r, 3r+3). A ReduceScatter over each 4-core group sums the
per-head output partials and hands each rank a 512-row sequence slice, on
which the core runs LN + FFN + residual. The host gathers the 8 [512, 768]
outputs.

Algebraic restructure (as before): folded per-head weights
    M_h = W_q[h] @ W_k[h].T        -> scores = x M x^T / sqrt(D)
    N_h = W_v[h] @ W_o[:, hD:+D].T -> out_h  = softmax_num @ (x N) / denom
with the softmax denominator produced by a ones column appended to u = x N.

fp8 acceleration (DoubleRow matmuls, 2 contraction rows per PE pass):
  * error-sensitive value-path matmuls (G/u projections, FFN1) run as
    3-term hi/lo split-fp8 (a_hi*b_hi + a_hi*b_lo + a_lo*b_hi), which is
    bf16-accurate at 0.75x the bf16 PE cost;
  * the score matmul runs single-fp8 (0.25x cost) - softmax tolerates the
    logit noise (adds ~1.3e-2 rel err, budget is 2e-2);
  * attn@u and FFN2 stay bf16 (split-operand production there costs more
    DVE/ACT time than the PE it saves).
Host pre-scales (powers of 2, folded into the exp/gelu activation scales and
the softmax-denominator ones column) keep every fp8 operand in e4m3 range.
"""

import math
from dataclasses import dataclass

import numpy as np
import ml_dtypes

P = 128
SL = 512  # q-chunk width (PSUM bank / matmul free-dim limit)

# host-side power-of-2 scales for fp8 operands
S_X = 8.0     # x
S_M = 128.0   # M = W_q W_k^T
S_N = 512.0   # N = W_v W_o_slice^T
S_W1 = 64.0   # ff_w1
C_G = 2.0 ** -7  # on-device scale when casting the G psum to fp8


@dataclass(frozen=True)
class Cfg:
    S: int = 2048          # sequence length
    D: int = 768           # model dim (= per-head dim here)
    FF: int = 3072         # FFN hidden dim
    HEADS: int = 3         # heads per core
    R: int = 4             # ranks per reduce-scatter group
    n_cores: int = 8

    @property
    def dch(self):
        return self.D // P

    @property
    def fch(self):
        return self.FF // P

    @property
    def qc(self):
        return self.S // SL

    @property
    def kt(self):
        return self.S // P

    @property
    def q_local(self):
        return self.S // self.R

    @property
    def qlt(self):
        return self.q_local // P


PHASES = []


def build_graph(cfg: Cfg, no_collective: bool = False):
    """no_collective=True replaces the ReduceScatter with a local DMA so the
    graph can run under the single-core TimelineSim for perf iteration."""
    import concourse.tile as tile
    from concourse import bacc, mybir
    from concourse.masks import make_identity

    f32 = mybir.dt.float32
    bf16 = mybir.dt.bfloat16
    f8 = mybir.dt.float8e4
    DR = mybir.MatmulPerfMode.DoubleRow
    S, D, FF = cfg.S, cfg.D, cfg.FF
    DCH, FCH, QC, KT, QLT = cfg.dch, cfg.fch, cfg.qc, cfg.kt, cfg.qlt
    DCH2 = DCH // 2  # fp8 DoubleRow pair-slices over the contraction dim
    HEADS, R = cfg.HEADS, cfg.R
    DP = SL // P  # k-tiles per q-chunk on the diagonal (4)
    # split the D free-dim into <=SL pieces for matmuls (PSUM bank limit)
    d_splits = [(s0, min(s0 + SL, D)) for s0 in range(0, D, SL)]
    # same for the u matrix, which has a trailing ones column (D+1 wide)
    u_splits = [(s0, min(s0 + SL, D + 1)) for s0 in range(0, D + 1, SL)]
    # exp scale: psum logits carry S_X (x) * S_M*S_X*C_G (g8) scale
    esc = 1.0 / (math.sqrt(D) * S_X * S_M * C_G * S_X)
    n_groups = cfg.n_cores // R
    replica_groups = [list(range(g * R, (g + 1) * R)) for g in range(n_groups)]

    nc = bacc.Bacc(
        "TRN2",
        target_bir_lowering=False,
        debug=False,
        enable_asserts=True,
        num_devices=cfg.n_cores,
    )

    # ---- I/O (per-core shards, pre-transposed / pre-scaled / fp8-split) ----
    x8hi_d = nc.dram_tensor("x8hi", [D, S], f8, kind="ExternalInput")  # (x[b].T)*S_X
    x8lo_d = nc.dram_tensor("x8lo", [D, S], f8, kind="ExternalInput")
    m8hi_d = nc.dram_tensor("m8hi", [HEADS, D, D], f8, kind="ExternalInput")
    m8lo_d = nc.dram_tensor("m8lo", [HEADS, D, D], f8, kind="ExternalInput")
    n8hi_d = nc.dram_tensor("n8hi", [HEADS, D, D], f8, kind="ExternalInput")
    n8lo_d = nc.dram_tensor("n8lo", [HEADS, D, D], f8, kind="ExternalInput")
    w1hi_d = nc.dram_tensor("w1hi", [D, FF], f8, kind="ExternalInput")  # ff_w1.T*S_W1
    w1lo_d = nc.dram_tensor("w1lo", [D, FF], f8, kind="ExternalInput")
    ff_w2_t = nc.dram_tensor("ff_w2_t", [FF, D], bf16, kind="ExternalInput")
    out_ext = nc.dram_tensor("out", [cfg.q_local, D], f32, kind="ExternalOutput")

    with tile.TileContext(nc) as tc:
        with (
            tc.tile_pool(name="consts", bufs=1) as consts,
            tc.tile_pool(name="big", bufs=1) as big,
            tc.tile_pool(name="wts", bufs=1) as wts,
            tc.tile_pool(name="attn", bufs=2) as attn_pool,
            tc.tile_pool(name="small", bufs=2) as small,
            tc.tile_pool(name="stage", bufs=2) as stage,
            tc.tile_pool(name="dram", bufs=1, space="DRAM") as dram_pool,
            tc.tile_pool(name="psA", bufs=4, space="PSUM") as psA,
            tc.tile_pool(name="psB", bufs=4, space="PSUM") as psB,
        ):
            # internal DRAM for the reduce-scatter
            rs_in = dram_pool.tile([S, D], f32, name="rs_in")
            rs_out = dram_pool.tile([cfg.q_local, D], f32, name="rs_out")

            # ---- constants ----
            # causal mask for the (narrowed) diagonal tiles:
            # mask0[kr, qc] = 1 where qc >= kr
            mask0 = consts.tile([P, SL], bf16, tag="mask", name="mask0")
            nc.gpsimd.memset(mask0, 1.0)
            nc.gpsimd.affine_select(
                out=mask0,
                in_=mask0,
                compare_op=mybir.AluOpType.is_ge,
                fill=0.0,
                base=0,
                pattern=[[1, SL]],
                channel_multiplier=-1,
            )
            identity = consts.tile([P, P], bf16, tag="ident", name="identity")
            make_identity(nc, identity)
            eps_col = consts.tile([P, 1], f32, tag="eps", name="eps_col")
            nc.vector.memset(eps_col, 1e-5)

            def load_head_weights(h, interleave_xt=False):
                mw_hi = wts.tile([P, DCH, D], f8, tag="mwhi", bufs=1, name=f"mwh{h}")
                mw_lo = wts.tile([P, DCH, D], f8, tag="mwlo", bufs=1, name=f"mwl{h}")
                nw_hi = wts.tile([P, DCH, D], f8, tag="nwhi", bufs=1, name=f"nwh{h}")
                nw_lo = wts.tile([P, DCH, D], f8, tag="nwlo", bufs=1, name=f"nwl{h}")
                mhi_src = m8hi_d.ap()[h].rearrange("(c p) e -> p c e", p=P)
                mlo_src = m8lo_d.ap()[h].rearrange("(c p) e -> p c e", p=P)
                if interleave_xt:
                    # critical path at startup: the first projection group
                    # needs m_w[:, :, 0:128] + x chunk 0 first; x goes on the
                    # scalar HWDGE queue (idle until the first exp) so the
                    # two queues fill SBUF in parallel
                    nc.sync.dma_start(mw_hi[:, :, 0:P], mhi_src[:, :, 0:P])
                    nc.sync.dma_start(mw_lo[:, :, 0:P], mlo_src[:, :, 0:P])
                    nc.scalar.dma_start(x8hi[:, :, 0:SL], xhi_src[:, :, 0:SL])
                    nc.scalar.dma_start(x8lo[:, :, 0:SL], xlo_src[:, :, 0:SL])
                    nc.sync.dma_start(mw_hi[:, :, P:D], mhi_src[:, :, P:D])
                    nc.sync.dma_start(mw_lo[:, :, P:D], mlo_src[:, :, P:D])
                    nc.sync.dma_start(nw_hi, n8hi_d.ap()[h].rearrange("(c p) e -> p c e", p=P))
                    nc.sync.dma_start(nw_lo, n8lo_d.ap()[h].rearrange("(c p) e -> p c e", p=P))
                else:
                    nc.sync.dma_start(mw_hi, mhi_src)
                    nc.sync.dma_start(mw_lo, mlo_src)
                    nc.sync.dma_start(nw_hi, n8hi_d.ap()[h].rearrange("(c p) e -> p c e", p=P))
                    nc.sync.dma_start(nw_lo, n8lo_d.ap()[h].rearrange("(c p) e -> p c e", p=P))
                return mw_hi, mw_lo, nw_hi, nw_lo

            x8hi = big.tile([P, DCH, S], f8, tag="xhi", name="x8hi")
            x8lo = big.tile([P, DCH, S], f8, tag="xlo", name="x8lo")
            xhi_src = x8hi_d.ap().rearrange("(c p) s -> p c s", p=P)
            xlo_src = x8lo_d.ap().rearrange("(c p) s -> p c s", p=P)
            head_weights = load_head_weights(0, interleave_xt=True)
            for sc in range(1, QC):
                nc.sync.dma_start(
                    x8hi[:, :, sc * SL:(sc + 1) * SL],
                    xhi_src[:, :, sc * SL:(sc + 1) * SL],
                )
                nc.sync.dma_start(
                    x8lo[:, :, sc * SL:(sc + 1) * SL],
                    xlo_src[:, :, sc * SL:(sc + 1) * SL],
                )

            def dr_accum(ps_ap, term_ops, col, cslice, first, last):
                """Emit the split-fp8 DoubleRow accumulation into ps_ap.

                term_ops: list of (stationary, moving) tile pairs; each
                contributes DCH2 DoubleRow matmuls (contraction pair-slices).
                col: stationary column slice; cslice: moving column slice.
                """
                n = len(term_ops) * DCH2
                i = 0
                for stat, mov in term_ops:
                    for pc in range(DCH2):
                        i += 1
                        nc.tensor.matmul(
                            ps_ap,
                            stat[:, 2 * pc:2 * pc + 2, col],
                            mov[:, 2 * pc:2 * pc + 2, cslice],
                            start=(first and i == 1),
                            stop=(last and i == n),
                            perf_mode=DR,
                            skip_group_check=True,
                        )

            def weave(a, b):
                """Emit closure lists a and b interleaved proportionally.

                Keeps the PE queue fed with independent work from both
                streams so neither's evacuation engine (ACT/DVE) stalls it.
                """
                na, nb = len(a), len(b)
                ia = ib = 0
                while ib < min(4, nb):
                    b[ib]()
                    ib += 1
                while ia < na or ib < nb:
                    if ib >= nb or (ia < na and ia * nb <= ib * na):
                        a[ia]()
                        ia += 1
                    else:
                        b[ib]()
                        ib += 1

            # per-head state: weights / gt8 / u_sb / es tiles
            hst = [dict() for _ in range(HEADS)]

            def head_stream(h):
                """Emit weight DMAs now; return closures computing gt8 (fp8),
                u (bf16), and the first q-chunk's scores for head h."""
                st = hst[h]
                st["w"] = head_weights if h == 0 else load_head_weights(h)
                st["gt8"] = big.tile([P, DCH, S], f8, tag="qt", name=f"gt{h}")
                st["u"] = big.tile([P, KT, D + 1], bf16, tag="v", bufs=2,
                                   name=f"u{h}")
                mw_hi, mw_lo, nw_hi, nw_lo = st["w"]
                cls = [lambda: nc.vector.memset(st["u"][:, :, D:D + 1], S_X * S_N)]

                def gproj(sc, ec):
                    def go():
                        ps = psA.tile([P, SL], f32, tag="psA", name="ps_proj")
                        dr_accum(
                            ps,
                            [(mw_hi, x8hi), (mw_hi, x8lo), (mw_lo, x8hi)],
                            slice(ec * P, (ec + 1) * P),
                            slice(sc * SL, (sc + 1) * SL),
                            True,
                            True,
                        )
                        nc.scalar.activation(
                            out=st["gt8"][:, ec, sc * SL:(sc + 1) * SL],
                            in_=ps,
                            func=mybir.ActivationFunctionType.Copy,
                            scale=C_G,
                        )
                    return go

                def uproj(kti):
                    def go():
                        for (e0, e1) in d_splits:
                            pv = psA.tile([P, SL], f32, tag="psA", name="pv")
                            dr_accum(
                                pv[:, : e1 - e0],
                                [(x8hi, nw_hi), (x8hi, nw_lo), (x8lo, nw_hi)],
                                slice(kti * P, (kti + 1) * P),
                                slice(e0, e1),
                                True,
                                True,
                            )
                            nc.vector.tensor_copy(
                                out=st["u"][:, kti, e0:e1], in_=pv[:, : e1 - e0]
                            )
                    return go

                cls += [gproj(0, ec) for ec in range(DCH)]
                cls += [uproj(kti) for kti in range(KT)]
                cls += scores_closures(h, 0)
                cls += [gproj(sc, ec) for sc in range(1, QC) for ec in range(DCH)]
                return cls

            def scores_closures(h, sc):
                """Closures: one per k-tile of the (h, sc) score pass
                (single-fp8 DR matmul -> exp -> diag mask on Pool)."""
                st = hst[h]
                n_kt = (sc + 1) * DP
                diag0 = sc * DP

                def mk(kti):
                    def go():
                        if kti == 0:
                            st[("es", sc)] = attn_pool.tile(
                                [P, n_kt, SL], bf16, tag="es", bufs=2,
                                name=f"es{h}_{sc}",
                            )
                        es_all = st[("es", sc)]
                        m = kti - diag0
                        o = m * P if m > 0 else 0
                        w = SL - o
                        st_ps = psA.tile([P, SL], f32, tag="psA", name="st_ps")
                        for pc in range(DCH2):
                            nc.tensor.matmul(
                                st_ps[:, :w],
                                x8hi[:, 2 * pc:2 * pc + 2, kti * P:(kti + 1) * P],
                                st["gt8"][:, 2 * pc:2 * pc + 2,
                                          sc * SL + o:(sc + 1) * SL],
                                start=(pc == 0),
                                stop=(pc == DCH2 - 1),
                                perf_mode=DR,
                                skip_group_check=True,
                            )
                        nc.scalar.activation(
                            out=es_all[:, kti, :w],
                            in_=st_ps[:, :w],
                            func=mybir.ActivationFunctionType.Exp,
                            scale=esc,
                        )
                        if m >= 0:
                            nc.gpsimd.tensor_mul(
                                out=es_all[:, kti, :w],
                                in0=es_all[:, kti, :w],
                                in1=mask0[:, :w],
                            )
                    return go

                return [mk(kti) for kti in range(n_kt)]

            def num_closures(h, sc):
                """Closures for the numerator+denominator pass of (h, sc):
                out'[q,:] = sum_k es^T u'. u carries a trailing S_X*S_N
                column, so col D is the softmax denominator at u's scale and
                normalization cancels it. Two q-subtiles at a time in PSUM."""
                st = hst[h]
                n_kt = (sc + 1) * DP
                diag0 = sc * DP
                ops = {}
                cls = []
                for half in range(DP // 2):
                    qls = (2 * half, 2 * half + 1)
                    # (ops keys are per-ql; each half uses fresh ql values)

                    def mk_mm(kti, qls=qls):
                        def go():
                            es_all = st[("es", sc)]
                            m = kti - diag0
                            o = m * P if m > 0 else 0
                            for ql in qls:
                                if m > ql:
                                    continue  # fully masked block
                                if ql not in ops:
                                    ops[ql] = [
                                        psB.tile([P, SL], f32, tag="psB",
                                                 name=f"o{ql}_{i}")
                                        for i in range(len(u_splits))
                                    ]
                                es_sl = es_all[:, kti, ql * P - o:(ql + 1) * P - o]
                                for op_t, (e0, e1) in zip(ops[ql], u_splits):
                                    nc.tensor.matmul(
                                        op_t[:, : e1 - e0],
                                        es_sl,
                                        st["u"][:, kti, e0:e1],
                                        start=(kti == 0),
                                        stop=(kti == diag0 + ql),
                                        skip_group_check=True,
                                    )
                        return go

                    def mk_fin(ql):
                        def go():
                            q0 = sc * SL + ql * P
                            last_e0 = u_splits[-1][0]
                            recd = small.tile([P, 1], f32, tag="recd", name="recd")
                            nc.vector.reciprocal(
                                out=recd,
                                in_=ops[ql][-1][:, D - last_e0:D - last_e0 + 1],
                            )
                            wo_stage = stage.tile(
                                [P, D], f32, tag="st768", bufs=2, name="wo_stage"
                            )
                            for op_t, (e0, e1) in zip(ops[ql], u_splits):
                                nc.vector.tensor_scalar_mul(
                                    out=wo_stage[:, e0:min(e1, D)],
                                    in0=op_t[:, : min(e1, D) - e0],
                                    scalar1=recd,
                                )
                            if h == 0:
                                nc.sync.dma_start(
                                    out=rs_in[q0:q0 + P, :], in_=wo_stage
                                )
                            else:
                                nc.gpsimd.dma_start(
                                    out=rs_in[q0:q0 + P, :],
                                    in_=wo_stage,
                                    accum_op=mybir.AluOpType.add,
                                )
                        return go

                    cls += [mk_mm(kti) for kti in range(n_kt)]
                    cls += [mk_fin(ql) for ql in qls]
                return cls

            # ---- pipelined emission: the exp-gated scores pass of the next
            # q-chunk (and the next head's projections) weave into the
            # PE-dense numerator pass of the current chunk ----
            PHASES.append(("start", nc.get_next_instruction_name()))
            for f in head_stream(0):
                f()
            ffw2 = wts.tile([P, FCH, D], bf16, tag="ffw2", name="ffw2")
            for h in range(HEADS):
                if h == HEADS - 1:
                    # DMA queues are idle during attention: preload ff_w2 now
                    nc.sync.dma_start(
                        ffw2, ff_w2_t.ap().rearrange("(c p) e -> p c e", p=P)
                    )
                for sc in range(QC):
                    if sc < QC - 1:
                        nxt = scores_closures(h, sc + 1)
                    elif h < HEADS - 1:
                        nxt = head_stream(h + 1)
                    else:
                        nxt = []
                    PHASES.append((f"num{h}.{sc}", nc.get_next_instruction_name()))
                    weave(num_closures(h, sc), nxt)

            PHASES.append(("rs", nc.get_next_instruction_name()))
            # ---- reduce-scatter: sum partials over the group, keep local rows ----
            if no_collective:
                nc.sync.dma_start(out=rs_out, in_=rs_in[: cfg.q_local, :])
            else:
                nc.gpsimd.collective_compute(
                    "ReduceScatter",
                    mybir.AluOpType.add,
                    replica_groups=replica_groups,
                    ins=[rs_in.opt()],
                    outs=[rs_out.opt()],
                )

            # ---- FFN on the local q_local rows ----
            # ff_w2 is resident (loaded during the attention phase)

            # residual rows, one q-tile per DMA so LN stats start early
            resid = big.tile([P, QLT, D], f32, tag="v", bufs=2, name="resid")
            resid_src = rs_out.rearrange("(t p) e -> p t e", p=P)
            for qt_i in range(QLT):
                nc.sync.dma_start(
                    resid[:, qt_i, :], resid_src[:, qt_i, :]
                )

            PHASES.append(("ln", nc.get_next_instruction_name()))
            # layernorm (no affine) -> ln^T fp8 hi/lo [d, q_local]
            lnThi = big.tile([P, DCH, cfg.q_local], f8, tag="lnhi", name="lnThi")
            lnTlo = big.tile([P, DCH, cfg.q_local], f8, tag="lnlo", name="lnTlo")
            ln_all = stage.tile([P, QLT, D], bf16, tag="ln_row", bufs=1, name="ln_all")
            for qt_i in range(QLT):
                x_row = resid[:, qt_i, :]
                sub = math.gcd(512, D)
                nsub = D // sub
                stats = small.tile([P, nsub, 6], f32, tag="stats", name="stats")
                for si in range(nsub):
                    nc.vector.bn_stats(
                        out=stats[:, si, :], in_=x_row[:, si * sub:(si + 1) * sub]
                    )
                mv = small.tile([P, 2], f32, tag="mv", name="mv")
                nc.vector.bn_aggr(out=mv, in_=stats)
                rstd = small.tile([P, 1], f32, tag="rstd", name="rstd")
                nc.scalar.activation(
                    out=rstd,
                    in_=mv[:, 1:2],
                    func=mybir.ActivationFunctionType.Sqrt,
                    bias=eps_col,
                    scale=1.0,
                )
                nc.vector.reciprocal(out=rstd, in_=rstd)
                nc.vector.tensor_scalar(
                    out=ln_all[:, qt_i, :],
                    in0=x_row,
                    scalar1=mv[:, 0:1],
                    scalar2=rstd,
                    op0=mybir.AluOpType.subtract,
                    op1=mybir.AluOpType.mult,
                )
            for qt_i in range(QLT):
                for dc in range(DCH):
                    tr_ps = psA.tile([P, P], bf16, tag="psA", name="tr_ps")
                    nc.tensor.transpose(
                        tr_ps, ln_all[:, qt_i, dc * P:(dc + 1) * P], identity
                    )
                    dst = slice(qt_i * P, (qt_i + 1) * P)
                    nc.scalar.activation(
                        out=lnThi[:, dc, dst],
                        in_=tr_ps,
                        func=mybir.ActivationFunctionType.Copy,
                        scale=1.0,
                    )
                    nc.vector.tensor_tensor(
                        out=lnTlo[:, dc, dst],
                        in0=tr_ps,
                        in1=lnThi[:, dc, dst],
                        op=mybir.AluOpType.subtract,
                    )

            PHASES.append(("ffn1", nc.get_next_instruction_name()))
            # h^T = gelu(ff_w1 @ ln^T)  [f, q_local] bf16 (split-fp8 matmul)
            hT = big.tile([P, FCH, cfg.q_local], bf16, tag="v", bufs=2, name="hT")
            for fc in range(FCH):
                w1c_hi = wts.tile([P, DCH, P], f8, tag="w1chi", bufs=8,
                                  name=f"w1ch{fc}")
                w1c_lo = wts.tile([P, DCH, P], f8, tag="w1clo", bufs=8,
                                  name=f"w1cl{fc}")
                # hi/lo on separate DMA queues so the stream keeps pace with
                # the 9-matmul PE burst per f-chunk
                nc.sync.dma_start(
                    w1c_hi,
                    w1hi_d.ap()[:, fc * P:(fc + 1) * P].rearrange(
                        "(c p) f -> p c f", p=P
                    ),
                )
                nc.scalar.dma_start(
                    w1c_lo,
                    w1lo_d.ap()[:, fc * P:(fc + 1) * P].rearrange(
                        "(c p) f -> p c f", p=P
                    ),
                )
                hp = psB.tile([P, SL], f32, tag="psB", name="hp")
                dr_accum(
                    hp,
                    [(w1c_hi, lnThi), (w1c_hi, lnTlo), (w1c_lo, lnThi)],
                    slice(0, P),
                    slice(0, cfg.q_local),
                    True,
                    True,
                )
                nc.scalar.activation(
                    out=hT[:, fc, :],
                    in_=hp,
                    func=mybir.ActivationFunctionType.Gelu,
                    scale=1.0 / S_W1,
                )

            PHASES.append(("ffn2", nc.get_next_instruction_name()))
            # y = h^T.T @ ff_w2^T + resid -> out
            for qt_i in range(QLT):
                yps = [
                    psA.tile([P, SL], f32, tag="psA", name=f"y{i}")
                    for i in range(len(d_splits))
                ]
                for fc in range(FCH):
                    for y_ps, (e0, e1) in zip(yps, d_splits):
                        nc.tensor.matmul(
                            y_ps[:, : e1 - e0],
                            hT[:, fc, qt_i * P:(qt_i + 1) * P],
                            ffw2[:, fc, e0:e1],
                            start=(fc == 0),
                            stop=(fc == FCH - 1),
                        )
                out_stage = stage.tile([P, D], f32, tag="st768", bufs=2, name="out_stage")
                for y_ps, (e0, e1) in zip(yps, d_splits):
                    nc.vector.tensor_add(
                        out=out_stage[:, e0:e1],
                        in0=y_ps[:, : e1 - e0],
                        in1=resid[:, qt_i, e0:e1],
                    )
                # halves on both queues: shortens the post-FFN drain tail
                h0 = D // 2
                nc.sync.dma_start(
                    out=out_ext.ap()[qt_i * P:(qt_i + 1) * P, 0:h0],
                    in_=out_stage[:, 0:h0],
                )
                nc.scalar.dma_start(
                    out=out_ext.ap()[qt_i * P:(qt_i + 1) * P, h0:D],
                    in_=out_stage[:, h0:D],
                )

    nc.compile()
    return nc


def shard_inputs(x, W_q, W_k, W_v, W_o, ff_w1, ff_w2, cfg: Cfg):
    f8 = ml_dtypes.float8_e4m3
    bf16 = ml_dtypes.bfloat16
    D = cfg.D

    def split8(a, scale):
        s = np.ascontiguousarray(a).astype(np.float32) * scale
        hi = s.astype(f8)
        lo = (s - hi.astype(np.float32)).astype(f8)
        return hi, lo

    w1hi, w1lo = split8(ff_w1.T, S_W1)
    ff2 = np.ascontiguousarray(ff_w2.T).astype(bf16)
    in_maps = []
    for c in range(cfg.n_cores):
        b, r = divmod(c, cfg.R)
        heads = range(cfg.HEADS * r, cfg.HEADS * (r + 1))
        # fold the per-head weight pairs on the host (fp32, then fp8 hi/lo):
        m = np.stack([W_q[h] @ W_k[h].T for h in heads])
        n = np.stack(
            [W_v[h] @ W_o[:, h * D:(h + 1) * D].T for h in heads]
        )
        mhi, mlo = split8(m, S_M)
        nhi, nlo = split8(n, S_N)
        xhi, xlo = split8(x[b].T, S_X)
        in_maps.append(
            {
                "x8hi": xhi, "x8lo": xlo,
                "m8hi": mhi, "m8lo": mlo,
                "n8hi": nhi, "n8lo": nlo,
                "w1hi": w1hi, "w1lo": w1lo,
                "ff_w2_t": ff2,
            }
        )
    return in_maps


def gather_outputs(results, cfg: Cfg, B):
    out = np.zeros((B, cfg.S, cfg.D), np.float32)
    for c in range(cfg.n_cores):
        b, r = divmod(c, cfg.R)
        out[b, cfg.q_local * r:cfg.q_local * (r + 1), :] = results[c]["out"]
    return out


def kernel(x, W_q, W_k, W_v, W_o, ff_w1, ff_w2):
    import sys

    if "/opt/trn_rl_repo" not in sys.path:
        sys.path.insert(0, "/opt/trn_rl_repo")
    from concourse.bass_utils import run_bass_kernel_spmd

    cfg = Cfg()
    nc = build_graph(cfg)
    in_maps = shard_inputs(x, W_q, W_k, W_v, W_o, ff_w1, ff_w2, cfg)
    res = run_bass_kernel_spmd(nc, in_maps, core_ids=list(range(cfg.n_cores)))
    return gather_outputs(res.results, cfg, x.shape[0])
